# revision 40
# baseline (speedup 1.0000x reference)
"""Trainium2 Bass kernel for nn_Attention_7584912245222.

Math (reference):
    hidden = tanh(memory @ Wh + (query @ Wq)[:, None, :])   # [B, T, D]
    s      = softmax(hidden @ v, axis=T)                    # [B, T]
    out    = einsum('btd,bt->bd', memory, s)                # [B, D]

Strategy: pure data-parallel over batch B=64 across 8 NeuronCores
(8 batches per core). Weights replicated. No collectives.

Device pipeline (per core, per batch b):
  - main GEMM in fp8-e4m3 DoubleRow mode (2 k-tiles of 128 contraction
    per instruction, 2x MACs/cycle vs bf16). Wh is host-prescaled by 32
    so its entries sit in e4m3's normal range; the 1/32 is folded into
    the tanh activation's `scale` operand. hidden.T is computed in
    [e(partitions), t(free)] orientation so the per-batch bias qvec[b][e]
    is a per-partition scalar fused into the PSUM->SBUF tanh on ScalarE.
  - v-weighting runs on VectorE:
      acc_hv[p, t] += h_m[p, t] * v[m*128+p]   (scalar_tensor_tensor)
    and the partition reduction s[t] = sum_p acc_hv[p, t] is done with 16
    tiny PE matmuls producing s directly TRANSPOSED as [t(part), 1].
  - softmax without max-subtraction (logits bounded, |s| < ~4): one Exp
    activation on the [128, KT] transposed tile, accum_out + one
    [128,1] @ ones matmul give the normalizer.
  - final weighted sum is SPLIT between VectorE f32 accumulation
      acc_d[p, d] += memN_j[p, d] * s_exp[j*128+p]
    (most j-tiles; Vector would otherwise idle behind the PE) and direct
    PE matmuls lhsT=sT16[:,j] rhs=memN_j accumulated in PSUM (last few
    j-tiles, plus ALL tiles for the last two batches so the kernel tail
    isn't gated by Vector's slower f32 read-modify-write). Both partial
    sums land in one PSUM group via the ones-reduction; the 1/Z scale is
    applied by the activation `scale` operand on the way out.

memN (for the final weighted sum) stays bf16: quantizing it to fp8
would put ~2.4% error directly on the output. GpSimd is useless for the
elementwise work here (no TensorScalarPtr opcode on Pool, shares its
SBUF port with VectorE) but serves as a third DMA queue (SWDGE) for
memN halves and non-critical startup loads.

Run-to-run variance: the chip's clock sits at ~2.3GHz or ~1.9GHz
per-run (DVFS lottery); exec is ~305us in fast mode, ~360us throttled.
"""

import sys

if "/opt/trn_rl_repo" not in sys.path:
    sys.path.insert(0, "/opt/trn_rl_repo")

import numpy as np
import ml_dtypes

import concourse.bass as bass
import concourse.tile as tile
from concourse import bacc, bass_isa, mybir
from concourse.bass_utils import run_bass_kernel_spmd

BF16 = ml_dtypes.bfloat16
F8 = ml_dtypes.float8_e4m3
WH_SCALE = 32.0


def _install_ntff_hook_shim():
    """This image's antenv lacks axon_hooks; inject it so bass_utils'
    trace path (taken when BASS_TRACE is set) doesn't ImportError."""
    try:
        import types

        if "antenv.axon_hooks" in sys.modules:
            return
        import antenv

        mod = types.ModuleType("antenv.axon_hooks")
        mod._hook = None
        mod.set_axon_ntff_profile_hook = lambda h: setattr(mod, "_hook", h)
        mod.get_axon_ntff_profile_hook = lambda: mod._hook
        sys.modules["antenv.axon_hooks"] = mod
        antenv.axon_hooks = mod
        try:
            from trn_agent_boot.trn_boot import _ntff_profile_via_ctypes

            mod._hook = _ntff_profile_via_ctypes("/opt/axon/libaxon_pjrt.so")
        except Exception:
            pass
    except Exception:
        pass


_install_ntff_hook_shim()

# Problem shapes (hardcoded per spec)
B, T, D, Q = 64, 2048, 1024, 1024
N_CORES = 8
BL = B // N_CORES  # batches per core


def build(nc, BL=BL, T=T, D=D, Q=Q):
    """Emit the per-core kernel into `nc`. Returns nc."""
    f32 = mybir.dt.float32
    bf16 = mybir.dt.bfloat16
    fp8 = mybir.dt.float8e4
    AF = mybir.ActivationFunctionType
    ALU = mybir.AluOpType
    DR = mybir.MatmulPerfMode.DoubleRow

    P = 128
    TC = min(512, T)          # t-chunk size for the main GEMM
    DC = min(512, D)          # d-chunk size for the final output
    KD = D // P               # d contraction tiles
    KD2 = KD // 2             # d contraction k-tile PAIRS (DoubleRow)
    ME = D // P               # e output tiles
    KQ = Q // P               # q contraction tiles
    NT = T // TC              # t chunks
    KT = T // P               # t contraction tiles (final sum)
    ND = D // DC              # output d chunks

    memT = nc.declare_dram_parameter("memT", [BL, NT, P, KD * TC], fp8, isOutput=False)
    memN = nc.declare_dram_parameter("memN", [BL, T, D], bf16, isOutput=False)
    wh = nc.declare_dram_parameter("Wh", [P, KD * D], fp8, isOutput=False)
    wq = nc.declare_dram_parameter("Wq", [P, KQ * D], bf16, isOutput=False)
    qryT = nc.declare_dram_parameter("qryT", [P, KQ * BL], bf16, isOutput=False)
    vT = nc.declare_dram_parameter("vT", [P, KD], f32, isOutput=False)
    out_ext = nc.declare_dram_parameter("out", [BL, D], f32, isOutput=True)

    with tile.TileContext(nc) as tc:
        from contextlib import ExitStack
        import functools

        with ExitStack() as ctx:
            const_pool = ctx.enter_context(tc.tile_pool(name="const", bufs=1))

            wh_sb = const_pool.tile([P, KD * D], fp8, tag="wh")
            v_sb = const_pool.tile([P, KD], f32, tag="v")
            qry_sb = const_pool.tile([P, KQ * BL], bf16, tag="qry")
            wq_sb = const_pool.tile([P, KQ * D], bf16, tag="wq")
            ones_sb = const_pool.tile([P, 1], bf16, tag="ones")
            nc.gpsimd.memset(ones_sb[:], 1.0)
            onesw_sb = const_pool.tile([P, 32], bf16, tag="onesw")
            nc.gpsimd.memset(onesw_sb[:], 1.0)
            ones32_sb = const_pool.tile([P, 1], f32, tag="ones32")
            nc.gpsimd.memset(ones32_sb[:], 1.0)
            qT_sb = const_pool.tile([P, ME * BL], f32, tag="qT")  # col = m*BL+b

            mT_pool = ctx.enter_context(tc.tile_pool(name="mT", bufs=2 * NT))

            def emit_mT_load(b, tiles, eng=None):
                # col layout within a chunk: k2 * (2*TC) + i*TC + t; one
                # tile PER CHUNK so GEMM dependencies are chunk-granular
                chunks = []
                for n in range(NT):
                    c_sb = mT_pool.tile([P, KD * TC], fp8, tag="mT",
                                        name=f"mT{b}_{n}")
                    (eng or nc.scalar).dma_start(c_sb[:], memT[b, n])
                    chunks.append(c_sb)
                tiles[b] = chunks
                return tiles

            mT_tiles = {}

            # Startup DMAs spread across the three DGE queues (SP/Act/Pool —
            # a single queue only sustains ~107 GB/s). wq goes FIRST, split
            # three ways: the compile-time scheduler interleaves the qT
            # matmuls into the GEMM stream, so a late wq stalls the whole
            # in-order PE queue. wh/memT0 stream in behind it.
            q1 = KQ * D // 4       # sync's share (it also carries wh)
            q2 = KQ * D * 5 // 8   # scalar/gpsimd boundary
            nc.sync.dma_start(qry_sb[:], qryT[:])
            nc.sync.dma_start(wq_sb[:, 0:q1], wq[:, 0:q1])
            nc.scalar.dma_start(wq_sb[:, q1:q2], wq[:, q1:q2])
            nc.gpsimd.dma_start(wq_sb[:, q2:], wq[:, q2:])
            nc.sync.dma_start(wh_sb[:], wh[:])
            # memT0: first chunks on Act queue, later chunks on Pool
            mT0 = []
            for n in range(NT):
                c_sb = mT_pool.tile([P, KD * TC], fp8, tag="mT",
                                    name=f"mT0_{n}")
                eng = nc.scalar if n < (NT + 1) // 2 else nc.gpsimd
                eng.dma_start(c_sb[:], memT[0, n])
                mT0.append(c_sb)
            mT_tiles[0] = mT0
            nc.sync.dma_start(v_sb[:], vT[:])
            # batch 1's memT right behind batch 0's on the same queue, so
            # batch 1's GEMM isn't paced by its own just-issued loads
            emit_mT_load(1, mT_tiles)

            def emit_qT(pq_pool, mlo, mhi):
                # bias vectors qvec[b] = query[b] @ Wq
                for m in range(mlo, mhi):
                    pq = pq_pool.tile([P, BL], f32, tag="pq", name=f"pq{m}")
                    for k in range(KQ):
                        nc.tensor.matmul(
                            pq[:],
                            lhsT=wq_sb[
                                :, m * KQ * P + k * P : m * KQ * P + (k + 1) * P
                            ],
                            rhs=qry_sb[:, k * BL : (k + 1) * BL],
                            start=(k == 0),
                            stop=(k == KQ - 1),
                        )
                    nc.scalar.copy(qT_sb[:, m * BL : (m + 1) * BL], pq[:])

            # PE warm-up: dummy matmuls during the startup DMA window flip
            # the HAM clock gate to 8/8 before real work; the qT phase sits
            # between the two halves, and the tail keeps the clock hot
            # until wh/memT0 land
            with (
                tc.tile_pool(name="wup", bufs=1) as wu_pool,
                tc.tile_pool(name="wupp", bufs=1, space="PSUM") as wup_pool,
            ):
                wu_sb = wu_pool.tile([P, 512], bf16, tag="wu")
                nc.gpsimd.memset(wu_sb[:], 0.0)
                wu_ps = wup_pool.tile([32, 512], f32, tag="wups")
                for i in range(22):
                    # first chunk ramps the clock before qT; the rest keeps
                    # it hot (and the PE fed) until wq/wh land
                    if i == 10:
                        with tc.tile_pool(name="pqp", bufs=1,
                                          space="PSUM") as pq_pool:
                            emit_qT(pq_pool, 0, ME)
                    nc.tensor.matmul(
                        wu_ps[:],
                        lhsT=onesw_sb[:],
                        rhs=wu_sb[:],
                        start=True,
                        stop=True,
                        skip_group_check=True,
                    )

            ph_pool = ctx.enter_context(tc.tile_pool(name="ph", bufs=4, space="PSUM"))
            pm_pool = ctx.enter_context(tc.tile_pool(name="pm", bufs=1, space="PSUM"))

            mN_pool = ctx.enter_context(tc.tile_pool(name="mN", bufs=4))
            h_pool = ctx.enter_context(tc.tile_pool(name="h", bufs=5))
            acc_pool = ctx.enter_context(tc.tile_pool(name="acc", bufs=2))
            accd_pool = ctx.enter_context(tc.tile_pool(name="accd", bufs=2))
            s_pool = ctx.enter_context(tc.tile_pool(name="s", bufs=2))
            sm_pool = ctx.enter_context(tc.tile_pool(name="sm", bufs=2))

            def make_phase3(b, acc_hv, mN_sb, n_pe_j=3):
                # the final weighted sum over KT j-tiles is split: VectorE
                # accumulates j in [0, J_v) (f32 stt), the PE handles the
                # last n_pe_j tiles as direct sT16 matmuls into the same
                # PSUM group that the ones-reduction of acc_d lands in.
                st = {}
                J_v = KT - n_pe_j
                kh = KT // 2

                def mnj(j):
                    return mN_sb[j // kh], (j % kh) * D

                def emit_reduce_exp():
                    # s (transposed): sT[:, j] = acc_hv[:, j*128:(j+1)*128].T @ ones
                    # (one spare column holds the Z scalar so pm stays 1 bank)
                    sT_ps = pm_pool.tile([P, KT + 1], f32, tag="sT", name=f"sT{b}")
                    st["sT_ps"] = sT_ps
                    for j in range(KT):
                        nc.tensor.matmul(
                            sT_ps[:, j : j + 1],
                            lhsT=acc_hv[:, j * P : (j + 1) * P],
                            rhs=ones_sb[:, 0:1],
                            start=True,
                            stop=True,
                        )
                    sT_exp = s_pool.tile([P, KT], f32, tag="sTe", name=f"sTe{b}")
                    partials = s_pool.tile([P, 1], f32, tag="par", name=f"par{b}")
                    nc.scalar.activation(
                        sT_exp[:], sT_ps[:, 0:KT], AF.Exp, accum_out=partials[:]
                    )
                    st["sT_exp"] = sT_exp
                    st["partials"] = partials
                    if n_pe_j > 0:
                        sT16 = s_pool.tile([P, KT], bf16, tag="sTe16",
                                           name=f"sTe16_{b}")
                        nc.scalar.copy(sT16[:], sT_exp[:])
                        st["sT16"] = sT16

                def emit_z():
                    rec = s_pool.tile([1, 1], f32, tag="rec", name=f"rec{b}")
                    zps = st["sT_ps"][0:1, KT : KT + 1]
                    nc.tensor.matmul(
                        zps,
                        lhsT=ones32_sb[:, 0:1],
                        rhs=st["partials"][:],
                        start=True,
                        stop=True,
                        skip_group_check=True,
                    )
                    nc.vector.reciprocal(rec[0:1, 0:1], zps)
                    st["rec"] = rec

                def emit_acc_part(j0, j1):
                    # acc_d[p, d] = sum_{j<J_v} memN_j[p, d] * s_exp[j*128+p]
                    if j0 >= J_v:
                        return
                    j1 = min(j1, J_v)
                    if j0 == 0:
                        st["acc_d"] = accd_pool.tile(
                            [P, D], f32, tag="accd", name=f"accd{b}"
                        )
                    acc_d = st["acc_d"]
                    for j in range(j0, j1):
                        mh, off = mnj(j)
                        if j == 0:
                            nc.vector.tensor_scalar_mul(
                                acc_d[:],
                                mh[:, off : off + D],
                                st["sT_exp"][:, 0:1],
                            )
                        else:
                            nc.vector.scalar_tensor_tensor(
                                acc_d[:],
                                mh[:, off : off + D],
                                st["sT_exp"][:, j : j + 1],
                                acc_d[:],
                                op0=ALU.mult,
                                op1=ALU.add,
                            )

                def emit_direct(n):
                    # PE-side partial weighted sum for j in [J_v, KT)
                    if n_pe_j == 0:
                        return
                    if n == 0:
                        st["ops"] = pm_pool.tile([1, D], f32, tag="ops",
                                                 name=f"ops{b}")
                    ops = st["ops"]
                    for j in range(J_v, KT):
                        mh, off = mnj(j)
                        nc.tensor.matmul(
                            ops[0:1, n * DC : (n + 1) * DC],
                            lhsT=st["sT16"][:, j : j + 1],
                            rhs=mh[:, off + n * DC : off + (n + 1) * DC],
                            start=(j == J_v),
                            stop=(J_v == 0 and j == KT - 1),
                            skip_group_check=True,
                        )

                def emit_out():
                    out_row = s_pool.tile([1, D], f32, tag="orow",
                                          name=f"orow{b}", bufs=1)
                    if "ops" not in st:
                        st["ops"] = pm_pool.tile([1, D], f32, tag="ops",
                                                 name=f"ops{b}")
                    ops = st["ops"]
                    if J_v > 0:
                        for n in range(ND):
                            nc.tensor.matmul(
                                ops[0:1, n * DC : (n + 1) * DC],
                                lhsT=ones32_sb[:, 0:1],
                                rhs=st["acc_d"][:, n * DC : (n + 1) * DC],
                                start=(n_pe_j == 0),
                                stop=True,
                                skip_group_check=True,
                            )
                    for n in range(ND):
                        nc.scalar.activation(
                            out_row[0:1, n * DC : (n + 1) * DC],
                            ops[0:1, n * DC : (n + 1) * DC],
                            AF.Copy,
                            scale=st["rec"][0:1, 0:1],
                        )
                    nc.sync.dma_start(out_ext[b : b + 1, :], out_row[:])

                def emit_directs():
                    emit_direct(0)
                    emit_direct(1)

                if J_v > 0:
                    qq = max(1, (J_v + 2) // 3)
                    parts = [(i, min(i + qq, J_v)) for i in range(0, J_v, qq)]
                    accfns = [
                        functools.partial(emit_acc_part, a, c) for a, c in parts
                    ]
                    # direct PE pieces only need sT16; schedule them early so
                    # the PE work spreads across the next batch's GEMM
                    first = accfns[0]
                    pieces = [
                        emit_reduce_exp,
                        lambda: (emit_z(), first()),
                        emit_directs,
                    ] + accfns[1:] + [emit_out]
                else:
                    pieces = [emit_reduce_exp, emit_z, emit_directs, emit_out]
                return pieces

            # phase3 pieces of batch b run at checkpoints inside batch b+1
            CHECKPOINTS = {2, 3, 4, 5, 6, 7}
            pending = []
            for b in range(BL):
                last = b == BL - 1
                if b not in mT_tiles:
                    emit_mT_load(b, mT_tiles)
                mT_sb = mT_tiles.pop(b)
                # two half-tiles so phase-3 j-tiles only wait on their half
                kh = KT // 2
                mN_sb = [
                    mN_pool.tile([P, kh * D], bf16, tag="mN", name=f"mN{b}_0"),
                    mN_pool.tile([P, (KT - kh) * D], bf16, tag="mN",
                                 name=f"mN{b}_1"),
                ]

                acc_hv = acc_pool.tile([P, T], bf16, tag="acc", name=f"acc{b}")

                for m in range(ME):
                    h_sb = h_pool.tile([P, T], bf16, tag="h", name=f"h{b}_{m}")
                    phs = []

                    for n in range(NT):
                        ph = ph_pool.tile([P, TC], f32, tag="ph", name=f"ph{b}_{m}_{n}")
                        phs.append(ph)
                        for k2 in range(KD2):
                            lhsT = wh_sb[
                                :, m * D + k2 * 2 * P : m * D + (k2 + 1) * 2 * P
                            ].rearrange("p (two e) -> p two e", two=2)
                            rhs = mT_sb[n][
                                :, k2 * 2 * TC : (k2 + 1) * 2 * TC
                            ].rearrange("p (two t) -> p two t", two=2)
                            nc.tensor.matmul(
                                ph[:],
                                lhsT=lhsT,
                                rhs=rhs,
                                start=(k2 == 0),
                                stop=(k2 == KD2 - 1),
                                perf_mode=DR,
                            )
                        nc.scalar.activation(
                            h_sb[:, n * TC : (n + 1) * TC],
                            phs[n][:],
                            AF.Tanh,
                            bias=qT_sb[:, m * BL + b : m * BL + b + 1],
                            scale=1.0 / WH_SCALE,
                        )
                    # v-weighted accumulation on VectorE; the last m-tile
                    # is chunked so acc_hv completes right after the last
                    # tanh chunk instead of one full-width op later
                    if m == ME - 1 and m > 0:
                        for n in range(NT):
                            sl = slice(n * TC, (n + 1) * TC)
                            nc.vector.scalar_tensor_tensor(
                                acc_hv[:, sl],
                                h_sb[:, sl],
                                v_sb[:, m : m + 1],
                                acc_hv[:, sl],
                                op0=ALU.mult,
                                op1=ALU.add,
                            )
                    elif m == 0:
                        nc.vector.tensor_scalar_mul(
                            acc_hv[:], h_sb[:], v_sb[:, 0:1]
                        )
                    else:
                        nc.vector.scalar_tensor_tensor(
                            acc_hv[:],
                            h_sb[:],
                            v_sb[:, m : m + 1],
                            acc_hv[:],
                            op0=ALU.mult,
                            op1=ALU.add,
                        )
                    # memN is first needed by phase-3 (during batch b+1);
                    # defer its DMA so it doesn't compete with the critical
                    # memT/weight loads at startup. Split across two queues:
                    # 4MB on one ~107GB/s queue wouldn't meet its deadline.
                    if m == min(3, ME - 1):
                        nc.sync.dma_start(
                            mN_sb[0][:].rearrange("p (k d) -> p k d", k=kh),
                            memN[b, 0 : kh * P].rearrange(
                                "(k p) d -> p k d", p=P
                            ),
                        )
                        nc.gpsimd.dma_start(
                            mN_sb[1][:].rearrange(
                                "p (k d) -> p k d", k=KT - kh
                            ),
                            memN[b, kh * P :].rearrange(
                                "(k p) d -> p k d", p=P
                            ),
                        )
                    # interleave previous batch's phase-3 between dense
                    # main-GEMM blocks
                    if m in CHECKPOINTS and pending:
                        pending.pop(0)()
                        while m == ME - 1 and pending:
                            pending.pop(0)()
                # prefetch batch b+2's memT now that batch b's slot is
                # almost free (avoids stalling the ScalarE stream early)
                if b + 2 < BL:
                    emit_mT_load(b + 2, mT_tiles)
                # flush any leftover phase-3 pieces (small-ME debug configs)
                for fn in pending:
                    fn()
                pending = []

                # last batches lean on the PE so the kernel tail isn't
                # gated by Vector's f32 accumulation; earlier batches give
                # Vector most of the j-tiles (PE is the busier engine)
                if b >= BL - 2:
                    n_pe_j = KT
                else:
                    n_pe_j = 3 * KT // 16
                pending = make_phase3(b, acc_hv, mN_sb, n_pe_j=n_pe_j)
                if last:
                    for fn in pending:
                        fn()
                    pending = []

    nc.compile()
    return nc


# ---------------------------------------------------------------------------
# Host side
# ---------------------------------------------------------------------------

_CACHED_NC = None


def _get_nc():
    global _CACHED_NC
    if _CACHED_NC is None:
        nc = bacc.Bacc("TRN2", target_bir_lowering=False, debug=False,
                       num_devices=N_CORES)
        _CACHED_NC = build(nc)
    return _CACHED_NC


def prep_in_maps(memory, query, Wh, Wq, v):
    """Shard + lay out inputs for the 8 cores (host-side transforms only)."""
    P = 128
    KQ = Q // P
    KD = D // P
    KD2 = KD // 2
    ME = D // P
    # DoubleRow k-pair layout: col = m*D + k2*256 + i*128 + e, holding
    # Wh[(2*k2+i)*128+p, m*128+e] * WH_SCALE in e4m3
    Wh_b = np.ascontiguousarray(
        (Wh * WH_SCALE)
        .reshape(KD2, 2, P, ME, P)
        .transpose(2, 3, 0, 1, 4)
        .reshape(P, KD * D)
        .astype(F8)
    )
    # m-major: col = m*KQ*128 + k*128 + c, so half-loads cover m-halves
    Wq_b = np.ascontiguousarray(
        Wq.reshape(KQ, P, ME, P).transpose(1, 2, 0, 3).reshape(P, KQ * D)
        .astype(BF16)
    )
    vT = np.ascontiguousarray(v[:, 0].reshape(KD, P).T.astype(np.float32))  # [128, KD]
    in_maps = []
    BLL = BL
    for c in range(N_CORES):
        sl = slice(c * BLL, (c + 1) * BLL)
        mem_c = memory[sl]
        # memT[b, n, p, k2*2*TC + i*TC + t] = mem[b, n*TC+t, (2*k2+i)*128+p]
        TC = min(512, T)
        NT = T // TC
        memT_c = np.ascontiguousarray(
            mem_c.reshape(BLL, NT, TC, KD2, 2, P)
            .transpose(0, 1, 5, 3, 4, 2)
            .reshape(BLL, NT, P, KD * TC)
            .astype(F8)
        )
        memN_c = np.ascontiguousarray(mem_c.astype(BF16))  # [BL, T, D]
        # qryT[p, k*BL+b] = query[b, k*128+p]  (exact SBUF layout)
        qryT_c = np.ascontiguousarray(
            query[sl].T.reshape(KQ, P, BLL).transpose(1, 0, 2).reshape(P, KQ * BLL)
            .astype(BF16)
        )
        in_maps.append(
            {
                "memT": memT_c,
                "memN": memN_c,
                "Wh": Wh_b,
                "Wq": Wq_b,
                "qryT": qryT_c,
                "vT": vT,
            }
        )
    return in_maps


def run(in_maps, trace=False, **kwargs):
    nc = _get_nc()
    return run_bass_kernel_spmd(
        nc, in_maps, list(range(N_CORES)), trace=trace, **kwargs
    )


def kernel(memory, query, Wh, Wq, v):
    in_maps = prep_in_maps(memory, query, Wh, Wq, v)
    res = run(in_maps)
    out = np.concatenate([res.results[c]["out"] for c in range(N_CORES)], axis=0)
    return out.astype(np.float32)
